# revision 21
# baseline (speedup 1.0000x reference)
"""Trainium2 Bass kernel for nn_DataTermLayer (data-term update of optical flow).

Math: the reference's bilinear warp feeds *normalized* coords in [-1,1] into a
pixel-space sampler, so the gather only touches I1[b, 0:3, 0:3] and the warp
value is piecewise-bilinear in nx = (2w+u)/511 - 1, ny = (2h+v)/511 - 1:

  warp = [nx>=0][ny>=0] * bilin(P, nx, ny)
  out_u = u + (0.1*I2 - 0.1*warp) * (I1[h+1,w]-I1[h,w])
  out_v = v + (0.1*I2 - 0.1*warp) * (I1[h,w+1]-I1[h,w])

Numerics: everything runs in bf16 (rel-err budget 2e-2, measured 2.3e-3); the
hosts pre-converts the f32 inputs to bf16 and pre-permutes them into
partition-major layout (one DMA per image/tensor with 4-8KB descriptor
lines), and the x>1/y>1 hat-basis correction terms are dropped (they matter
only in the last row/col, error ~1e-4): warp == A0 + B0*nx + ny*(A1 + B1*nx)
with per-image coefficients folded with -0.1 on the host, applied as ACT
scale/bias APs.  Warp masks are applied only on the ambiguous strips
(cols 252..258, rows 128..255 of rb1 + rows 256..258); elsewhere the mask is
provably constant given |flow| < 6 (host-asserted).

Structure per image (layout [p=row%128, rb=row//128, w]):
 - row-gradient g1 on the idle PE: stationary matrix (subdiag-shift - I), so
   PSUM holds i1[r+1]-i1[r] directly; a second accumulating matmul fixes
   partition 127 from the next block (or zeroes the last row);
 - col-gradient g2 as a same-tile shifted TT (2x DVE mode);
 - dt = 0.1*I2 on ACT, warp t0/t1 on ACT, assembly/masks on DVE;
 - corr = dt * {g1(from PSUM), g2} interleaved via strided-dst TTs, one dense
   in-place pair add fl += corr, stores split in halves across both HWDGE
   queues.

Engine rules (measured): DVE bf16 dense TT 0.59 ns/elem (2x), TS 0.33 (4x),
strided ~1.2-1.4; ACT 0.98 always; Pool poisons DVE when running elementwise
concurrently (only memsets + SWDGE issue go there); per-DMA-queue throughput
is latency-bound (~10 descriptors in flight), so loads rotate across the
sync/scalar/gpsimd queues.  Emission is stage-major across the 4 images
(software pipelining across the in-order engine streams).

Sharding: pure data-parallel, 4 images per core across 8 cores; output is
stored bf16 partition-major and un-permuted/upcast on the host.
"""
import sys

sys.path.insert(0, "/opt/trn_rl_repo")

import numpy as np
import ml_dtypes

import concourse.bass as bass
import concourse.mybir as mybir
from concourse.bass_utils import run_bass_kernel_spmd
from concourse.tile import TileContext

F32 = mybir.dt.float32
BF16 = mybir.dt.bfloat16
ALU = mybir.AluOpType
ACTF = mybir.ActivationFunctionType

R511 = float(np.float32(1.0) / np.float32(511.0))
WZ = 252          # first possibly-warped column (2*251 + umax < 511)
NW = 512 - WZ     # warp-region columns
XSTRIP = 7        # cols WZ..WZ+6 have an ambiguous x-mask (2w+u vs 511)


def build_nc(n_imgs: int = 4):
    W, H, NRB = 512, 512, 4
    FD = NRB * W          # free elems per partition for one [512,512] image
    nc = bass.Bass()

    # partition-major layouts (host pre-permutes), all images contiguous
    # per partition so each tensor is ONE dma with 16-32KB descriptor lines
    I1 = nc.dram_tensor("I1", [128, n_imgs * NRB * W], BF16, kind="ExternalInput")
    I2 = nc.dram_tensor("I2", [128, n_imgs * NRB * W], BF16, kind="ExternalInput")
    FL = nc.dram_tensor("FL", [128, n_imgs * NRB * W * 2], BF16, kind="ExternalInput")
    # gx: (2w)*r511 - 1 for w in [WZ,512), tiled x3 rb; bf16
    GX = nc.dram_tensor("GX", [128, 3 * NW], BF16, kind="ExternalInput")
    # per-partition f32 consts: col rb: (2*(128*rb+p))*r511 - 1; cols 4..:
    # per-image warp coeffs (-0.1-scaled): 4*b+{0:A0,1:B0,2:A1,3:B1}
    CC = nc.dram_tensor("CC", [128, 128], F32, kind="ExternalInput")
    SH = nc.dram_tensor("SH", [128, 3 * 128], BF16, kind="ExternalInput")
    OUT = nc.dram_tensor("OUT", [n_imgs, 128, NRB * W * 2], BF16, kind="ExternalOutput")

    with TileContext(nc) as tc:
        with (
            tc.tile_pool(name="stat", bufs=1) as pstat,
            tc.tile_pool(name="pin", bufs=3) as pin,
            tc.tile_pool(name="ptmp", bufs=3) as ptmp,
            tc.tile_pool(name="pps", bufs=2, space="PSUM") as pps,
        ):
            sh = pstat.tile([128, 3 * 128], BF16)
            nc.gpsimd.dma_start(sh[:], SH[:])
            gx = pstat.tile([128, 3 * NW], BF16)
            nc.gpsimd.dma_start(gx[:], GX[:])
            gx3 = gx[:].rearrange("p (rb w) -> p rb w", rb=3)
            cc = pstat.tile([128, 128], F32)
            nc.gpsimd.dma_start(cc[:], CC[:])

            def cC(j):
                return cc[:, j : j + 1]

            # ============ per-image loads, queues rotated per tensor ======
            # (per-queue DMA is latency-bound at ~10 in-flight descriptors,
            # so concurrency across the three DGE queues is what matters;
            # separate tiles per image keep the dependency tracking sharp)
            i1ts, i2ts, flts = [], [], []
            # i1 is what the pipeline needs first: spread the four loads
            # over all three DGE queues so images 0-2 land concurrently
            i1engs = [nc.sync, nc.gpsimd, nc.scalar, nc.sync]
            for b in range(n_imgs):
                i1t = pin.tile([128, FD], BF16, tag="i1", bufs=4, name=f"i1_{b}")
                i1engs[b].dma_start(i1t[:], I1[:, b * FD : (b + 1) * FD])
                i1ts.append(i1t)
            for b in range(n_imgs):
                i2t = pin.tile([128, FD], BF16, tag="i2", bufs=4, name=f"i2_{b}")
                nc.gpsimd.dma_start(i2t[:], I2[:, b * FD : (b + 1) * FD])
                i2ts.append(i2t)
                flt = pin.tile([128, FD * 2], BF16, tag="fl", bufs=4, name=f"fl_{b}")
                fleng = nc.scalar if b % 2 == 0 else nc.sync
                fleng.dma_start(flt[:], FL[:, b * FD * 2 : (b + 1) * FD * 2])
                flts.append(flt)
            i1s = [t[:] for t in i1ts]
            i2s = [t[:] for t in i2ts]
            fls = [t[:] for t in flts]
            flvs = [f.rearrange("p (rb w c) -> p rb w c", rb=NRB, c=2) for f in fls]
            psums = []
            for b in range(n_imgs):
                # Row-gradient g1 on the (idle) PE: stationary matrix is
                # (subdiag-shift - identity), so psum[p] = i1[row+1] - i1[row]
                # directly; the second matmul accumulates the next block's
                # row 0 (or row 511 itself for rb=3, zeroing that gradient)
                # into partition 127.
                i1v = i1s[b].rearrange("p (rb w) -> p rb w", rb=NRB)
                ps = pps.tile([128, FD], F32, tag="ps", name=f"ps_{b}")
                for rb in range(NRB):
                    psb = ps[:, 512 * rb : 512 * (rb + 1)]
                    nc.tensor.matmul(psb, sh[:, 0:128], i1v[:, rb, :],
                                     start=True, stop=False)
                    bnd = 128 if rb < 3 else 256
                    nc.tensor.matmul(
                        psb, sh[:, bnd : bnd + 128],
                        i1v[:, min(rb + 1, 3), :], start=False, stop=True,
                    )
                psums.append(ps)

            # stage A: gradients (DVE) + g2 edge memsets (Pool)
            g2s, dts = [], []
            for b in range(n_imgs):
                i1v = i1s[b].rearrange("p (rb w) -> p rb w", rb=NRB)
                g2 = ptmp.tile([128, FD], BF16, tag="g2", bufs=4, name=f"g2_{b}")
                g2v = g2[:].rearrange("p (rb w) -> p rb w", rb=NRB)
                nc.vector.tensor_tensor(
                    g2v[:, :, 0:511], i1v[:, :, 1:512], i1v[:, :, 0:511],
                    ALU.subtract,
                )
                nc.gpsimd.memset(g2v[:, :, 511:512], 0.0)
                g2s.append(g2)
            # stage B: ACT producers (dt, uw, y)
            uws, ys = [], []
            for b in range(n_imgs):
                dt = ptmp.tile([128, FD], BF16, tag="dt", bufs=4, name=f"dt_{b}")
                nc.scalar.activation(dt[:], i2s[b], ACTF.Copy, bias=0.0, scale=0.1)
                dts.append(dt)
                uw = ptmp.tile([128, 3, NW], BF16, tag="uw", bufs=2, name=f"uw_{b}")
                nc.scalar.activation(
                    uw[:], flvs[b][:, 1:4, WZ:, 0], ACTF.Copy, bias=0.0, scale=R511
                )
                uws.append(uw)
                y = ptmp.tile([128, 3, NW], BF16, tag="y", bufs=3, name=f"y_{b}")
                for rb in range(1, 4):
                    nc.scalar.activation(
                        y[:, rb - 1, :], flvs[b][:, rb, WZ:, 1], ACTF.Identity,
                        bias=cC(rb), scale=R511,
                    )
                ys.append(y)

            # stage C: x (DVE)
            xs = []
            for b in range(n_imgs):
                x = ptmp.tile([128, 3, NW], BF16, tag="x", bufs=3, name=f"x_{b}")
                nc.vector.tensor_tensor(x[:], uws[b][:], gx3[:], ALU.add)
                xs.append(x)

            # stage D: t0/t1 (ACT)
            t0s, t1s = [], []
            for b in range(n_imgs):
                t0 = ptmp.tile([128, 3, NW], BF16, tag="t0", bufs=3, name=f"t0_{b}")
                nc.scalar.activation(
                    t0[:], xs[b][:], ACTF.Identity,
                    bias=cC(4 + 4 * b + 0), scale=cC(4 + 4 * b + 1),
                )
                t1 = ptmp.tile([128, 3, NW], BF16, tag="t1", bufs=3, name=f"t1_{b}")
                nc.scalar.activation(
                    t1[:], xs[b][:], ACTF.Identity,
                    bias=cC(4 + 4 * b + 2), scale=cC(4 + 4 * b + 3),
                )
                t0s.append(t0)
                t1s.append(t1)

            # stage E+F per image: warp assembly, masks, dt, flow update, store
            for b in range(n_imgs):
                t0, t1, x, y = t0s[b], t1s[b], xs[b], ys[b]
                nc.vector.tensor_tensor(t1[:], y[:], t1[:], ALU.mult)
                nc.vector.tensor_tensor(t0[:], t0[:], t1[:], ALU.add)
                nc.vector.scalar_tensor_tensor(
                    t0[:, :, 0:XSTRIP], x[:, :, 0:XSTRIP], 0.0,
                    t0[:, :, 0:XSTRIP], ALU.is_ge, ALU.mult,
                )
                nc.vector.scalar_tensor_tensor(
                    t0[:, 0:1, :], y[:, 0:1, :], 0.0, t0[:, 0:1, :],
                    ALU.is_ge, ALU.mult,
                )
                nc.vector.scalar_tensor_tensor(
                    t0[0:3, 1:2, :], y[0:3, 1:2, :], 0.0, t0[0:3, 1:2, :],
                    ALU.is_ge, ALU.mult,
                )
                dtv = dts[b][:].rearrange("p (rb w) -> p rb w", rb=NRB)
                nc.vector.tensor_tensor(
                    dtv[:, 1:4, WZ:], dtv[:, 1:4, WZ:], t0[:], ALU.add
                )
                corr = ptmp.tile([128, FD * 2], BF16, tag="corr", bufs=2,
                                 name=f"corr_{b}")
                corr2 = corr[:].rearrange("p (q c) -> p q c", c=2)
                nc.vector.tensor_tensor(
                    corr2[:, :, 0], dts[b][:], psums[b][:], ALU.mult
                )
                nc.vector.tensor_tensor(corr2[:, :, 1], dts[b][:], g2s[b][:], ALU.mult)
                for hv in range(2):
                    sl = slice(hv * FD, (hv + 1) * FD)
                    nc.vector.tensor_tensor(
                        fls[b][:, sl], fls[b][:, sl], corr[:, sl], ALU.add
                    )
                    eng = nc.scalar if (2 * b + hv) % 2 == 0 else nc.sync
                    eng.dma_start(OUT[b, :, sl], fls[b][:, sl])
    legalize_single_wait(nc)
    return nc


# ---------------------------------------------------------------------------
# Post-pass: this walrus build encodes a single sync-wait slot per TPB
# instruction. Tile's sem assignment can emit 2+ waits on one instruction;
# hoist all but the last wait onto same-engine EventSemaphore carriers placed
# immediately before it (the sequencer then waits sequentially, which is
# semantically identical).
def legalize_single_wait(nc):
    import bass_rust

    capped = {
        mybir.EngineType.Activation,
        mybir.EngineType.DVE,
        mybir.EngineType.Pool,
        mybir.EngineType.PE,
        mybir.EngineType.SP,
    }
    exempt = {"EventSemaphore", "NoOp", "TriggerDma"}
    n = 0
    for fn in nc.m.functions:
        for blk in fn.blocks:
            insts = blk.instructions  # live list
            rebuilt = []
            changed = False
            for inst in list(insts):
                si = inst.sync_info
                waits = list(si.on_wait) if si is not None else []
                if (
                    len(waits) > 1
                    and inst.engine in capped
                    and str(inst.opcode) not in exempt
                ):
                    for w in waits[:-1]:
                        ev = mybir.InstEventSemaphore(
                            name=f"waitcarrier_{inst.name}_{n}", ins=[], outs=[]
                        )
                        ev.engine = inst.engine
                        ev.sync_info = bass_rust.SyncInfo(
                            on_wait=[w], on_update=[]
                        )
                        rebuilt.append(ev)
                        n += 1
                    inst.sync_info = bass_rust.SyncInfo(
                        on_wait=[waits[-1]], on_update=list(si.on_update)
                    )
                    changed = True
                rebuilt.append(inst)
            if changed:
                insts[:] = rebuilt
    return n


def host_consts(I1c: np.ndarray, n_imgs: int) -> np.ndarray:
    """[128, 4 + 4*n] f32: col rb: (2*(128*rb+p))*r511 - 1; then per-image
    -0.1-scaled bilinear coeffs A0,B0,A1,B1 of the corner patch."""
    f = np.float32
    cc = np.zeros((128, 128), dtype=np.float32)
    p = np.arange(128, dtype=np.float64)
    for rb in range(4):
        cc[:, rb] = (2.0 * (128.0 * rb + p)) / 511.0 - 1.0
    for b in range(n_imgs):
        P = I1c[b, 0:3, 0:3].astype(np.float64)
        a0 = P[0, 0]
        b0 = P[0, 1] - P[0, 0]
        a1 = P[1, 0] - P[0, 0]
        b1 = P[1, 1] - P[1, 0] - P[0, 1] + P[0, 0]
        cc[:, 4 + 4 * b + 0] = f(-0.1 * a0)
        cc[:, 4 + 4 * b + 1] = f(-0.1 * b0)
        cc[:, 4 + 4 * b + 2] = f(-0.1 * a1)
        cc[:, 4 + 4 * b + 3] = f(-0.1 * b1)
    return cc


def host_sh() -> np.ndarray:
    sh = np.zeros((128, 3 * 128), dtype=np.float32)
    for p in range(127):
        sh[p + 1, p] = 1.0          # shift: psum[p] = i1[p+1] ...
    for p in range(128):
        sh[p, p] -= 1.0             # ... minus i1[p]: psum = g1 directly
    sh[0, 128 + 127] = 1.0          # boundary: psum[127] += rhs[0]
    sh[127, 256 + 127] = 1.0        # rb3 boundary: psum[127] += rhs[127]
    return sh.astype(ml_dtypes.bfloat16)


def _pack(img: np.ndarray) -> np.ndarray:
    """[n, 512, 512(,c)] -> partition-major bf16 [n, 128, 4*512(*c)]."""
    n = img.shape[0]
    x = img.reshape(n, 4, 128, -1)
    x = np.ascontiguousarray(x.transpose(0, 2, 1, 3)).reshape(n, 128, -1)
    return x.astype(ml_dtypes.bfloat16)


def host_gx() -> np.ndarray:
    w = np.arange(WZ, 512, dtype=np.float64)
    gx = (2.0 * w) / 511.0 - 1.0
    return np.tile(gx.astype(ml_dtypes.bfloat16), (128, 3))


_NC = None


def _get_nc():
    global _NC
    if _NC is None:
        _NC = build_nc(4)
    return _NC


def run(I1, I2, flow, trace=False, **kw):
    I1 = np.asarray(I1)
    I2 = np.asarray(I2)
    flow = np.asarray(flow)
    assert float(np.abs(flow).max()) < 6.0, "flow magnitude exceeds mask-strip bound"
    nc = _get_nc()
    gx = host_gx()
    sh = host_sh()
    in_maps = []
    per = I1.shape[0] // 8
    for c in range(8):
        sl = slice(c * per, (c + 1) * per)
        i1c = np.ascontiguousarray(I1[sl, :, :, 0], dtype=np.float32)
        in_maps.append(
            {
                "I1": np.ascontiguousarray(
                    _pack(i1c).transpose(1, 0, 2).reshape(128, -1)
                ),
                "I2": np.ascontiguousarray(
                    _pack(I2[sl, :, :, 0]).transpose(1, 0, 2).reshape(128, -1)
                ),
                "FL": np.ascontiguousarray(
                    _pack(flow[sl]).transpose(1, 0, 2).reshape(128, -1)
                ),
                "GX": gx,
                "CC": host_consts(i1c, per),
                "SH": sh,
            }
        )
    res = run_bass_kernel_spmd(nc, in_maps, list(range(8)), trace=trace, **kw)
    outs = []
    for r in res.results:
        o = r["OUT"].astype(np.float32)  # [per, 128, 4*512*2]
        o = o.reshape(per, 128, 4, 512, 2).transpose(0, 2, 1, 3, 4)
        outs.append(o.reshape(per, 512, 512, 2))
    out = np.concatenate(outs, axis=0)
    return out, res


def kernel(I1, I2, flow):
    out, _ = run(I1, I2, flow)
    return out.astype(np.float32)


# revision 22
# speedup vs baseline: 1.0646x; 1.0646x over previous
"""Trainium2 Bass kernel for nn_DataTermLayer (data-term update of optical flow).

Math: the reference's bilinear warp feeds *normalized* coords in [-1,1] into a
pixel-space sampler, so the gather only touches I1[b, 0:3, 0:3] and the warp
value is piecewise-bilinear in nx = (2w+u)/511 - 1, ny = (2h+v)/511 - 1:

  warp = [nx>=0][ny>=0] * bilin(P, nx, ny)
  out_u = u + (0.1*I2 - 0.1*warp) * (I1[h+1,w]-I1[h,w])
  out_v = v + (0.1*I2 - 0.1*warp) * (I1[h,w+1]-I1[h,w])

Numerics: everything runs in bf16 (rel-err budget 2e-2, measured 2.3e-3); the
hosts pre-converts the f32 inputs to bf16 and pre-permutes them into
partition-major layout (one DMA per image/tensor with 4-8KB descriptor
lines), and the x>1/y>1 hat-basis correction terms are dropped (they matter
only in the last row/col, error ~1e-4): warp == A0 + B0*nx + ny*(A1 + B1*nx)
with per-image coefficients folded with -0.1 on the host, applied as ACT
scale/bias APs.  Warp masks are applied only on the ambiguous strips
(cols 252..258, rows 128..255 of rb1 + rows 256..258); elsewhere the mask is
provably constant given |flow| < 6 (host-asserted).

Structure per image (layout [p=row%128, rb=row//128, w]):
 - row-gradient g1 on the idle PE: stationary matrix (subdiag-shift - I), so
   PSUM holds i1[r+1]-i1[r] directly; a second accumulating matmul fixes
   partition 127 from the next block (or zeroes the last row);
 - col-gradient g2 as a same-tile shifted TT (2x DVE mode);
 - dt = 0.1*I2 on ACT, warp t0/t1 on ACT, assembly/masks on DVE;
 - corr = dt * {g1(from PSUM), g2} interleaved via strided-dst TTs, one dense
   in-place pair add fl += corr, stores split in halves across both HWDGE
   queues.

Engine rules (measured): DVE bf16 dense TT 0.59 ns/elem (2x), TS 0.33 (4x),
strided ~1.2-1.4; ACT 0.98 always; Pool poisons DVE when running elementwise
concurrently (only memsets + SWDGE issue go there); per-DMA-queue throughput
is latency-bound (~10 descriptors in flight), so loads rotate across the
sync/scalar/gpsimd queues.  Emission is stage-major across the 4 images
(software pipelining across the in-order engine streams).

Sharding: pure data-parallel, 4 images per core across 8 cores; output is
stored bf16 partition-major and un-permuted/upcast on the host.
"""
import sys

sys.path.insert(0, "/opt/trn_rl_repo")

import numpy as np
import ml_dtypes

import concourse.bass as bass
import concourse.mybir as mybir
from concourse.bass_utils import run_bass_kernel_spmd
from concourse.tile import TileContext

F32 = mybir.dt.float32
BF16 = mybir.dt.bfloat16
ALU = mybir.AluOpType
ACTF = mybir.ActivationFunctionType

R511 = float(np.float32(1.0) / np.float32(511.0))
WZ = 252          # first possibly-warped column (2*251 + umax < 511)
NW = 512 - WZ     # warp-region columns
XSTRIP = 7        # cols WZ..WZ+6 have an ambiguous x-mask (2w+u vs 511)


def build_nc(n_imgs: int = 4):
    W, H, NRB = 512, 512, 4
    FD = NRB * W          # free elems per partition for one [512,512] image
    nc = bass.Bass()

    # partition-major layouts (host pre-permutes), all images contiguous
    # per partition so each tensor is ONE dma with 16-32KB descriptor lines
    I1 = nc.dram_tensor("I1", [128, n_imgs * NRB * W], BF16, kind="ExternalInput")
    I2 = nc.dram_tensor("I2", [128, n_imgs * NRB * W], BF16, kind="ExternalInput")
    FL = nc.dram_tensor("FL", [128, n_imgs * NRB * W * 2], BF16, kind="ExternalInput")
    # gx: (2w)*r511 - 1 for w in [WZ,512), tiled x3 rb; bf16
    GX = nc.dram_tensor("GX", [128, 3 * NW], BF16, kind="ExternalInput")
    # per-partition f32 consts: col rb: (2*(128*rb+p))*r511 - 1; cols 4..:
    # per-image warp coeffs (-0.1-scaled): 4*b+{0:A0,1:B0,2:A1,3:B1}
    CC = nc.dram_tensor("CC", [128, 128], F32, kind="ExternalInput")
    SH = nc.dram_tensor("SH", [128, 3 * 128], BF16, kind="ExternalInput")
    OUT = nc.dram_tensor("OUT", [n_imgs, 128, NRB * W * 2], BF16, kind="ExternalOutput")

    with TileContext(nc) as tc:
        with (
            tc.tile_pool(name="stat", bufs=1) as pstat,
            tc.tile_pool(name="pin", bufs=3) as pin,
            tc.tile_pool(name="ptmp", bufs=3) as ptmp,
            tc.tile_pool(name="pps", bufs=2, space="PSUM") as pps,
        ):
            sh = pstat.tile([128, 3 * 128], BF16)
            nc.gpsimd.dma_start(sh[:], SH[:])
            gx = pstat.tile([128, 3 * NW], BF16)
            nc.gpsimd.dma_start(gx[:], GX[:])
            gx3 = gx[:].rearrange("p (rb w) -> p rb w", rb=3)
            cc = pstat.tile([128, 128], F32)
            nc.gpsimd.dma_start(cc[:], CC[:])

            def cC(j):
                return cc[:, j : j + 1]

            # ============ per-image loads, queues rotated per tensor ======
            # (per-queue DMA is latency-bound at ~10 in-flight descriptors,
            # so concurrency across the three DGE queues is what matters;
            # separate tiles per image keep the dependency tracking sharp)
            i1ts, i2ts, flts = [], [], []
            for b in range(n_imgs):
                i1t = pin.tile([128, FD], BF16, tag="i1", bufs=4, name=f"i1_{b}")
                nc.sync.dma_start(i1t[:], I1[:, b * FD : (b + 1) * FD])
                i1ts.append(i1t)
            for b in range(n_imgs):
                i2t = pin.tile([128, FD], BF16, tag="i2", bufs=4, name=f"i2_{b}")
                nc.gpsimd.dma_start(i2t[:], I2[:, b * FD : (b + 1) * FD])
                i2ts.append(i2t)
                flt = pin.tile([128, FD * 2], BF16, tag="fl", bufs=4, name=f"fl_{b}")
                fleng = nc.scalar if b % 2 == 0 else nc.sync
                fleng.dma_start(flt[:], FL[:, b * FD * 2 : (b + 1) * FD * 2])
                flts.append(flt)
            i1s = [t[:] for t in i1ts]
            i2s = [t[:] for t in i2ts]
            fls = [t[:] for t in flts]
            flvs = [f.rearrange("p (rb w c) -> p rb w c", rb=NRB, c=2) for f in fls]
            psums = []
            for b in range(n_imgs):
                # Row-gradient g1 on the (idle) PE: stationary matrix is
                # (subdiag-shift - identity), so psum[p] = i1[row+1] - i1[row]
                # directly; the second matmul accumulates the next block's
                # row 0 (or row 511 itself for rb=3, zeroing that gradient)
                # into partition 127.
                i1v = i1s[b].rearrange("p (rb w) -> p rb w", rb=NRB)
                ps = pps.tile([128, FD], F32, tag="ps", name=f"ps_{b}")
                for rb in range(NRB):
                    psb = ps[:, 512 * rb : 512 * (rb + 1)]
                    nc.tensor.matmul(psb, sh[:, 0:128], i1v[:, rb, :],
                                     start=True, stop=False)
                    bnd = 128 if rb < 3 else 256
                    nc.tensor.matmul(
                        psb, sh[:, bnd : bnd + 128],
                        i1v[:, min(rb + 1, 3), :], start=False, stop=True,
                    )
                psums.append(ps)

            # stage A: gradients (DVE) + g2 edge memsets (Pool)
            g2s, dts = [], []
            for b in range(n_imgs):
                i1v = i1s[b].rearrange("p (rb w) -> p rb w", rb=NRB)
                g2 = ptmp.tile([128, FD], BF16, tag="g2", bufs=4, name=f"g2_{b}")
                g2v = g2[:].rearrange("p (rb w) -> p rb w", rb=NRB)
                nc.vector.tensor_tensor(
                    g2v[:, :, 0:511], i1v[:, :, 1:512], i1v[:, :, 0:511],
                    ALU.subtract,
                )
                nc.gpsimd.memset(g2v[:, :, 511:512], 0.0)
                g2s.append(g2)
            # stage B: ACT producers (dt, uw, y)
            uws, ys = [], []
            for b in range(n_imgs):
                dt = ptmp.tile([128, FD], BF16, tag="dt", bufs=4, name=f"dt_{b}")
                nc.scalar.activation(dt[:], i2s[b], ACTF.Copy, bias=0.0, scale=0.1)
                dts.append(dt)
                uw = ptmp.tile([128, 3, NW], BF16, tag="uw", bufs=2, name=f"uw_{b}")
                nc.scalar.activation(
                    uw[:], flvs[b][:, 1:4, WZ:, 0], ACTF.Copy, bias=0.0, scale=R511
                )
                uws.append(uw)
                y = ptmp.tile([128, 3, NW], BF16, tag="y", bufs=3, name=f"y_{b}")
                for rb in range(1, 4):
                    nc.scalar.activation(
                        y[:, rb - 1, :], flvs[b][:, rb, WZ:, 1], ACTF.Identity,
                        bias=cC(rb), scale=R511,
                    )
                ys.append(y)

            # stage C: x (DVE)
            xs = []
            for b in range(n_imgs):
                x = ptmp.tile([128, 3, NW], BF16, tag="x", bufs=3, name=f"x_{b}")
                nc.vector.tensor_tensor(x[:], uws[b][:], gx3[:], ALU.add)
                xs.append(x)

            # stage D: t0/t1 (ACT)
            t0s, t1s = [], []
            for b in range(n_imgs):
                t0 = ptmp.tile([128, 3, NW], BF16, tag="t0", bufs=3, name=f"t0_{b}")
                nc.scalar.activation(
                    t0[:], xs[b][:], ACTF.Identity,
                    bias=cC(4 + 4 * b + 0), scale=cC(4 + 4 * b + 1),
                )
                t1 = ptmp.tile([128, 3, NW], BF16, tag="t1", bufs=3, name=f"t1_{b}")
                nc.scalar.activation(
                    t1[:], xs[b][:], ACTF.Identity,
                    bias=cC(4 + 4 * b + 2), scale=cC(4 + 4 * b + 3),
                )
                t0s.append(t0)
                t1s.append(t1)

            # stage E+F per image: warp assembly, masks, dt, flow update, store
            for b in range(n_imgs):
                t0, t1, x, y = t0s[b], t1s[b], xs[b], ys[b]
                nc.vector.tensor_tensor(t1[:], y[:], t1[:], ALU.mult)
                nc.vector.tensor_tensor(t0[:], t0[:], t1[:], ALU.add)
                nc.vector.scalar_tensor_tensor(
                    t0[:, :, 0:XSTRIP], x[:, :, 0:XSTRIP], 0.0,
                    t0[:, :, 0:XSTRIP], ALU.is_ge, ALU.mult,
                )
                nc.vector.scalar_tensor_tensor(
                    t0[:, 0:1, :], y[:, 0:1, :], 0.0, t0[:, 0:1, :],
                    ALU.is_ge, ALU.mult,
                )
                nc.vector.scalar_tensor_tensor(
                    t0[0:3, 1:2, :], y[0:3, 1:2, :], 0.0, t0[0:3, 1:2, :],
                    ALU.is_ge, ALU.mult,
                )
                dtv = dts[b][:].rearrange("p (rb w) -> p rb w", rb=NRB)
                nc.vector.tensor_tensor(
                    dtv[:, 1:4, WZ:], dtv[:, 1:4, WZ:], t0[:], ALU.add
                )
                corr = ptmp.tile([128, FD * 2], BF16, tag="corr", bufs=2,
                                 name=f"corr_{b}")
                corr2 = corr[:].rearrange("p (q c) -> p q c", c=2)
                nc.vector.tensor_tensor(
                    corr2[:, :, 0], dts[b][:], psums[b][:], ALU.mult
                )
                nc.vector.tensor_tensor(corr2[:, :, 1], dts[b][:], g2s[b][:], ALU.mult)
                for hv in range(2):
                    sl = slice(hv * FD, (hv + 1) * FD)
                    nc.vector.tensor_tensor(
                        fls[b][:, sl], fls[b][:, sl], corr[:, sl], ALU.add
                    )
                    eng = nc.scalar if (2 * b + hv) % 2 == 0 else nc.sync
                    eng.dma_start(OUT[b, :, sl], fls[b][:, sl])
    legalize_single_wait(nc)
    return nc


# ---------------------------------------------------------------------------
# Post-pass: this walrus build encodes a single sync-wait slot per TPB
# instruction. Tile's sem assignment can emit 2+ waits on one instruction;
# hoist all but the last wait onto same-engine EventSemaphore carriers placed
# immediately before it (the sequencer then waits sequentially, which is
# semantically identical).
def legalize_single_wait(nc):
    import bass_rust

    capped = {
        mybir.EngineType.Activation,
        mybir.EngineType.DVE,
        mybir.EngineType.Pool,
        mybir.EngineType.PE,
        mybir.EngineType.SP,
    }
    exempt = {"EventSemaphore", "NoOp", "TriggerDma"}
    n = 0
    for fn in nc.m.functions:
        for blk in fn.blocks:
            insts = blk.instructions  # live list
            rebuilt = []
            changed = False
            for inst in list(insts):
                si = inst.sync_info
                waits = list(si.on_wait) if si is not None else []
                if (
                    len(waits) > 1
                    and inst.engine in capped
                    and str(inst.opcode) not in exempt
                ):
                    for w in waits[:-1]:
                        ev = mybir.InstEventSemaphore(
                            name=f"waitcarrier_{inst.name}_{n}", ins=[], outs=[]
                        )
                        ev.engine = inst.engine
                        ev.sync_info = bass_rust.SyncInfo(
                            on_wait=[w], on_update=[]
                        )
                        rebuilt.append(ev)
                        n += 1
                    inst.sync_info = bass_rust.SyncInfo(
                        on_wait=[waits[-1]], on_update=list(si.on_update)
                    )
                    changed = True
                rebuilt.append(inst)
            if changed:
                insts[:] = rebuilt
    return n


def host_consts(I1c: np.ndarray, n_imgs: int) -> np.ndarray:
    """[128, 4 + 4*n] f32: col rb: (2*(128*rb+p))*r511 - 1; then per-image
    -0.1-scaled bilinear coeffs A0,B0,A1,B1 of the corner patch."""
    f = np.float32
    cc = np.zeros((128, 128), dtype=np.float32)
    p = np.arange(128, dtype=np.float64)
    for rb in range(4):
        cc[:, rb] = (2.0 * (128.0 * rb + p)) / 511.0 - 1.0
    for b in range(n_imgs):
        P = I1c[b, 0:3, 0:3].astype(np.float64)
        a0 = P[0, 0]
        b0 = P[0, 1] - P[0, 0]
        a1 = P[1, 0] - P[0, 0]
        b1 = P[1, 1] - P[1, 0] - P[0, 1] + P[0, 0]
        cc[:, 4 + 4 * b + 0] = f(-0.1 * a0)
        cc[:, 4 + 4 * b + 1] = f(-0.1 * b0)
        cc[:, 4 + 4 * b + 2] = f(-0.1 * a1)
        cc[:, 4 + 4 * b + 3] = f(-0.1 * b1)
    return cc


def host_sh() -> np.ndarray:
    sh = np.zeros((128, 3 * 128), dtype=np.float32)
    for p in range(127):
        sh[p + 1, p] = 1.0          # shift: psum[p] = i1[p+1] ...
    for p in range(128):
        sh[p, p] -= 1.0             # ... minus i1[p]: psum = g1 directly
    sh[0, 128 + 127] = 1.0          # boundary: psum[127] += rhs[0]
    sh[127, 256 + 127] = 1.0        # rb3 boundary: psum[127] += rhs[127]
    return sh.astype(ml_dtypes.bfloat16)


def _pack(img: np.ndarray) -> np.ndarray:
    """[n, 512, 512(,c)] -> partition-major bf16 [n, 128, 4*512(*c)]."""
    n = img.shape[0]
    x = img.reshape(n, 4, 128, -1)
    x = np.ascontiguousarray(x.transpose(0, 2, 1, 3)).reshape(n, 128, -1)
    return x.astype(ml_dtypes.bfloat16)


def host_gx() -> np.ndarray:
    w = np.arange(WZ, 512, dtype=np.float64)
    gx = (2.0 * w) / 511.0 - 1.0
    return np.tile(gx.astype(ml_dtypes.bfloat16), (128, 3))


_NC = None


def _get_nc():
    global _NC
    if _NC is None:
        _NC = build_nc(4)
    return _NC


def run(I1, I2, flow, trace=False, **kw):
    I1 = np.asarray(I1)
    I2 = np.asarray(I2)
    flow = np.asarray(flow)
    assert float(np.abs(flow).max()) < 6.0, "flow magnitude exceeds mask-strip bound"
    nc = _get_nc()
    gx = host_gx()
    sh = host_sh()
    in_maps = []
    per = I1.shape[0] // 8
    for c in range(8):
        sl = slice(c * per, (c + 1) * per)
        i1c = np.ascontiguousarray(I1[sl, :, :, 0], dtype=np.float32)
        in_maps.append(
            {
                "I1": np.ascontiguousarray(
                    _pack(i1c).transpose(1, 0, 2).reshape(128, -1)
                ),
                "I2": np.ascontiguousarray(
                    _pack(I2[sl, :, :, 0]).transpose(1, 0, 2).reshape(128, -1)
                ),
                "FL": np.ascontiguousarray(
                    _pack(flow[sl]).transpose(1, 0, 2).reshape(128, -1)
                ),
                "GX": gx,
                "CC": host_consts(i1c, per),
                "SH": sh,
            }
        )
    res = run_bass_kernel_spmd(nc, in_maps, list(range(8)), trace=trace, **kw)
    outs = []
    for r in res.results:
        o = r["OUT"].astype(np.float32)  # [per, 128, 4*512*2]
        o = o.reshape(per, 128, 4, 512, 2).transpose(0, 2, 1, 3, 4)
        outs.append(o.reshape(per, 512, 512, 2))
    out = np.concatenate(outs, axis=0)
    return out, res


def kernel(I1, I2, flow):
    out, _ = run(I1, I2, flow)
    return out.astype(np.float32)
